# revision 4
# baseline (speedup 1.0000x reference)
"""
BiRNN Trainium2 kernel (8 NeuronCores, SPMD) — fp8 DoubleRow edition.

Problem: x:[64,512,64], bidirectional sigmoid RNN with H=1024, out O=512.
    xp = x @ Wx + bx                      (per time step)
    f_t = sigmoid(xp_t + f_{t-1} @ Ws + bs)   forward scan
    b_t = sigmoid(xp_t + b_{t+1} @ Ws + bs)   backward scan
    out = (f + b) @ Wout + bout

Strategy: speculative time-sharding. The map h -> sigmoid(xp + h@Ws + bs)
is strongly contractive (~0.35x/step), so chunks of the trajectory can be
recomputed from a junk state after W warmup steps. Each core runs TWO
independent 32-step time chunks (2 x 128 chains: 64 fwd + 64 bwd batch,
both directions fused), software-pipelined so one chunk's scan matmuls
fill the PE while the other chunk's tanh->transpose->copy tail cooks.
True-boundary chains (core 0 fwd, core 7 set-B bwd) are walled to
h ~ 0 (s = -1) during warmup and healed by an exact f32r fold matmul.

The numerics run a tanh/fp8 reformulation that cuts PE streaming 3x:
  s = 2h - 1 = tanh(pre/2):
      pre = x@Wx + [bx + bs + colsum(Ws)/2] + s_prev @ (Ws/2)
      out = (s_f + s_b) @ (Wout/2) + colsum(Wout) + bout
  All recurrent/projection matmuls are fp8e4m3 x fp8e4m3 DoubleRow
  (2 k-tiles/matmul, 0.5 cycles/row):
    - scan: per 256-col group, 4 DR matmuls contract all 8 h-chunks of
      q8(s) vs q8(Ws/2); a 5th DR pair is the input projection (q8(x)
      rows + split-bias rows + wall row).
    - projection: the f/b pair sum rides the two DR k-slots for free
      (slot0 = fwd tile, slot1 = bwd tile, both vs q8(Wout/2)); psum
      accumulates (f+b)@Wo8 exactly over an exact f32r bias-row opener.
      Output pairs are mirror-matched (times 15-p, 16+p) so each pair
      completes in a single slot and projections spread evenly.
  tanh on ScalarE (scale=0.5, fp8 out, bit-exact vs ml_dtypes emulation);
  fp8 PE-transposes (element-step-2 psum layout per walrus); one DVE
  copy per chunk-pair compacts them; pair routing on Pool. HW keeps fp8
  subnormals (probed): predicted rel err ~1.2e-2 vs the 2e-2 gate.
"""

import sys
from contextlib import ExitStack

import ml_dtypes
import numpy as np

if "/opt/trn_rl_repo" not in sys.path:
    sys.path.insert(0, "/opt/trn_rl_repo")

import concourse.bass as bass  # noqa: E402
import concourse.mybir as mybir  # noqa: E402
import concourse.tile as tile  # noqa: E402
from concourse import bacc  # noqa: E402
from concourse.bass_utils import run_bass_kernel_spmd  # noqa: E402
from concourse.masks import make_identity  # noqa: E402

F32 = mybir.dt.float32
F32R = mybir.dt.float32r
F16 = mybir.dt.float16
FP8 = mybir.dt.float8e4
F8NP = ml_dtypes.float8_e4m3
DR = mybir.MatmulPerfMode.DoubleRow

B, T, I, H, O = 64, 512, 64, 1024, 512
NCORES = 8
import os as _os
NSETS = int(_os.environ.get("K_NSETS", "4"))   # interleaved chunks per core
W = int(_os.environ.get("K_W", "3"))           # speculative warmup steps
NTH = int(_os.environ.get("K_NTH", "2"))       # tanh ops per step
NPC = int(_os.environ.get("K_NPC", "1"))       # DVE piece copies per step
PF = int(_os.environ.get("K_PF", "2"))         # xt prefetch distance (slots)
ODMA = _os.environ.get("K_ODMA", "scalar")     # out-store DMA queue
PHASE = _os.environ.get("K_PHASE", "interleave")  # slot emission order
TR = _os.environ.get("K_TR", "pe")             # state transpose: pe | dma(XBAR)
LAG = int(_os.environ.get("K_LAG", "1"))       # tail lags scan by N set-phases
C = T // (NCORES * NSETS)  # time-chunk per set
S = W + C                # slots per set
KC = H // 128            # 8 contraction chunks
NP2 = C // 2             # output pairs per set
NPAIR = NSETS * NP2      # output pairs per core
HALF = C // 2

_BUILD_CACHE = None


def _round_fp22(a):
    """Round fp32 array to FP22 (e8m13) — the float32r matmul input format."""
    u = np.ascontiguousarray(a, np.float32).view(np.uint32)
    u = (u + 0x200) & np.uint32(0xFFFFFC00)
    return u.view(np.float32)


def _q8(a):
    return np.asarray(a, np.float32).astype(F8NP)


def _build_program():
    """Build + compile the (SPMD-uniform) Bass program once."""
    global _BUILD_CACHE
    if _BUILD_CACHE is not None:
        return _BUILD_CACHE

    nc = bacc.Bacc("TRN2", target_bir_lowering=False, debug=False,
                   num_devices=NCORES)

    xt_d = nc.dram_tensor("xt", [NSETS, S, 128, 2, 128], FP8,
                          kind="ExternalInput").ap()
    ws_d = nc.dram_tensor("ws", [128, 4, 2, H], FP8, kind="ExternalInput").ap()
    wsx_d = nc.dram_tensor("wsx", [128, 2, H], FP8, kind="ExternalInput").ap()
    wo_d = nc.dram_tensor("wo", [128, KC, 2, O], FP8, kind="ExternalInput").ap()
    fold_d = nc.dram_tensor("fold", [128, NSETS, H], F32R,
                            kind="ExternalInput").ap()
    bb_d = nc.dram_tensor("bb", [128, O], F32, kind="ExternalInput").ap()
    out_d = nc.dram_tensor("out", [NPAIR, 128, O], F32,
                           kind="ExternalOutput").ap()

    with tile.TileContext(nc) as tc, ExitStack() as ctx:
        const = ctx.enter_context(tc.tile_pool(name="const", bufs=1))
        wsx_s = const.tile([128, 2, H], FP8)
        nc.sync.dma_start(wsx_s[:], wsx_d[:])
        ws_s = const.tile([128, 4, 2, H], FP8)
        nc.sync.dma_start(ws_s[:], ws_d[:])
        init_s = const.tile([128, KC, 128], FP8)
        nc.gpsimd.memset(init_s[:], 0.0)
        ident = const.tile([128, 128], F32)
        make_identity(nc, ident[:])
        ident_r = const.tile([128, 128], F32R)
        nc.scalar.copy(ident_r[:], ident[:])
        ident8 = const.tile([128, 128], FP8)
        nc.scalar.copy(ident8[:], ident[:])
        wo_s = const.tile([128, KC, 2, O], FP8)
        fold_s = const.tile([128, NSETS, H], F32R)
        bb_s = const.tile([128, O], F32)

        def emit_late_consts():
            nc.sync.dma_start(wo_s[:], wo_d[:])
            nc.sync.dma_start(fold_s[:], fold_d[:])
            nc.sync.dma_start(bb_s[:], bb_d[:])

        xt_pool = ctx.enter_context(tc.tile_pool(name="xt", bufs=8))
        pre_pool = ctx.enter_context(
            tc.tile_pool(name="pre", bufs=(3 if TR == "dma" else 2 if NTH == 1 else int(_os.environ.get("K_PREB", "5"))), space="PSUM"))
        th_pool = ctx.enter_context(tc.tile_pool(name="th", bufs=10))
        if TR == "pe":
            tr_pool = ctx.enter_context(
                tc.tile_pool(name="tr", bufs=1, space="PSUM"))
        else:
            sT16_pool = ctx.enter_context(tc.tile_pool(name="sT16", bufs=4))
        sT_pool = ctx.enter_context(tc.tile_pool(name="sT", bufs=6))
        fbp_pool = ctx.enter_context(tc.tile_pool(name="fbp", bufs=1))
        po_pool = ctx.enter_context(
            tc.tile_pool(name="po", bufs=int(_os.environ.get("K_POB", "2")), space="PSUM"))
        st_pool = ctx.enter_context(tc.tile_pool(name="st", bufs=3))

        fbp_t = [[fbp_pool.tile([128, 2, KC, 128], FP8, name=f"fbp{e}_{p}")
                  for p in range(NP2)] for e in range(NSETS)]

        pending = []
        pending_next = []

        def emit_xt_dma(e, slot):
            xt_t = xt_pool.tile([128, 2, 128], FP8, tag="xt")
            if TR == "dma":
                nc.scalar.dma_start(xt_t[:], xt_d[e, slot])
            else:
                nc.sync.dma_start(xt_t[:], xt_d[e, slot])
            return xt_t

        def _fused_proj(pair_t, pr):
            def emit():
                po_t = po_pool.tile([128, O], F32, tag="po")
                for kc in range(KC):
                    nc.tensor.matmul(po_t[:],
                                     pair_t[:, :, kc, :],
                                     wo_s[:, kc, :, :],
                                     start=(kc == 0), stop=(kc == KC - 1),
                                     perf_mode=DR)
                st_t = st_pool.tile([128, O], F32, tag="st")
                nc.vector.tensor_add(st_t[:], po_t[:], bb_s[:])
                if ODMA == "scalar":
                    nc.scalar.dma_start(out_d[pr], st_t[:])
                else:
                    nc.sync.dma_start(out_d[pr], st_t[:])
            return emit

        sT_prev = [init_s] * NSETS
        xt_q = [[emit_xt_dma(e, sl) for sl in range(min(PF, S))]
                for e in range(NSETS)]
        pending = []
        pending_next = []

        h_qs = [None] * NSETS
        pre_ts = [None] * NSETS

        def emit_scan(e, s):
            # input-projection DR matmuls open the 512-col (psum-bank
            # aligned) accumulation groups; then j-major scan matmuls.
            xt_t = xt_q[e].pop(0)
            if s + PF < S:
                xt_q[e].append(emit_xt_dma(e, s + PF))
            if TR == "dma" or NTH == 1:
                pre_w = pre_pool.tile([128, H], F32, tag="pre", name="prew")
                pre_g = [pre_w[:, bass.ts(0, 512)], pre_w[:, bass.ts(1, 512)]]
            else:
                pre_g = [pre_pool.tile([128, 512], F32, tag="pre",
                                       name=f"pre{gg}")
                         for gg in range(2)]
            for g in range(2):
                nc.tensor.matmul(pre_g[g],
                                 xt_t[:],
                                 wsx_s[:, :, bass.ts(g, 512)],
                                 start=True, stop=(s == 0), perf_mode=DR)
            if s == W:
                for g in range(2):
                    nc.tensor.matmul(pre_g[g],
                                     ident_r[:],
                                     fold_s[:, e, bass.ts(g, 512)],
                                     start=False, stop=False)
            if s > 0:
                # (slot 0 state is exactly zero: skip its scan matmuls)
                for j in range(4):
                    for g in range(2):
                        nc.tensor.matmul(pre_g[g],
                                         sT_prev[e][:, 2 * j:2 * j + 2, :],
                                         ws_s[:, j, :, bass.ts(g, 512)],
                                         start=False, stop=(j == 3),
                                         perf_mode=DR)

            if TR == "dma":
                hq = th_pool.tile([128, H], F16, tag="th", name="th16")
                nc.scalar.activation(hq[:], pre_w[:],
                                     mybir.ActivationFunctionType.Tanh,
                                     scale=0.5)
                h_q = [hq]
            elif NTH == 1:
                hq = th_pool.tile([128, H], FP8, tag="th", name="thw")
                nc.scalar.activation(hq[:], pre_w[:],
                                     mybir.ActivationFunctionType.Tanh,
                                     scale=0.5)
                h_q = [hq[:, bass.ts(0, 512)], hq[:, bass.ts(1, 512)]]
            else:
                h_q = []
                for nq in range(2):
                    hq = th_pool.tile([128, 512], FP8, tag="th")
                    nc.scalar.activation(hq[:], pre_g[nq][:],
                                         mybir.ActivationFunctionType.Tanh,
                                         scale=0.5)
                    h_q.append(hq)
            h_qs[e] = h_q

        def emit_tail(e, s):
            h_q = h_qs[e]
            if TR == "dma":
                sT16 = sT16_pool.tile([128, KC, 128], F16, tag="sT16")
                nc.sync.dma_start_transpose(sT16[:], h_q[0][:])
                sT_new = sT_pool.tile([128, KC, 128], FP8, tag="sT")
                nc.vector.tensor_copy(sT_new[:], sT16[:])
                emit_route(e, s, sT_new)
                sT_prev[e] = sT_new
                return
            tr_t = tr_pool.tile([128, KC, 256], FP8, tag="tr")
            sT_new = sT_pool.tile([128, KC, 128], FP8, tag="sT")
            cpc = KC // NPC            # chunks per DVE piece copy
            for kc in range(KC):
                qi, qo = divmod(kc, 4)
                nc.tensor.matmul(
                    tr_t[:, kc, 0:256:2],
                    h_q[qi][:, bass.ts(qo, 128)],
                    ident8[:],
                    is_transpose=True,
                    start=(kc == 0), stop=(kc == KC - 1))
                if kc % cpc == cpc - 1:
                    j = kc // cpc
                    nc.vector.tensor_copy(
                        sT_new[:, j * cpc:(j + 1) * cpc, :],
                        tr_t[:, j * cpc:(j + 1) * cpc, 0:256:2])

            emit_route(e, s, sT_new)
            sT_prev[e] = sT_new

        def emit_route(e, s, sT_new):
            if s < W:
                return
            tl = s - W
            # mirror pairing: pair p <-> times (HALF-1-p, HALF+p); fwd
            # state of time tl and bwd state of time C-1-tl both land
            # in pair p, opposite column halves / direction slots.
            p = HALF - 1 - tl if tl < HALF else tl - HALF
            cf = 0 if tl < HALF else 1
            dst = fbp_t[e][p]
            base = dst[:, 0, :, bass.ts(cf, 64)]
            nc.gpsimd.tensor_copy(base, sT_new[:, :, 0:64])
            nc.gpsimd.tensor_copy(
                dst[:, 1, :, bass.ts(1 - cf, 64)],
                sT_new[:, :, 64:128])
            if tl >= HALF:
                pending_next.append(
                    _fused_proj(fbp_t[e][p], e * NP2 + p))

        tailq = []
        for s in range(S):
            if PHASE == "split":
                for e in range(NSETS):
                    emit_scan(e, s)
                for fn in pending:
                    fn()
                pending = pending_next
                pending_next = []
                for e in range(NSETS):
                    emit_tail(e, s)
            else:
                q = pending
                pending = []
                for e in range(NSETS):
                    emit_scan(e, s)
                    if q:
                        q.pop(0)()
                    if LAG:
                        tailq.append((e, s))
                        if len(tailq) > LAG:
                            ee, ss = tailq.pop(0)
                            emit_tail(ee, ss)
                    else:
                        emit_tail(e, s)
                pending = q + pending_next
                pending_next = []
            if s == 0:
                emit_late_consts()
        for ee, ss in tailq:
            emit_tail(ee, ss)
        for fn in pending + pending_next:
            fn()

    nc.compile()
    _BUILD_CACHE = nc
    return nc


def _prepare_inputs(x, h0_f, h0_b, Wx, bx, Ws, bs, Wout, bout):
    """Host-side data marshaling: per-core input dicts."""
    x = np.ascontiguousarray(np.asarray(x, np.float32))
    h0_f = np.asarray(h0_f, np.float64)
    h0_b = np.asarray(h0_b, np.float64)
    Wx = np.asarray(Wx, np.float32)
    bx = np.asarray(bx, np.float64)
    Ws = np.asarray(Ws, np.float64)
    bs = np.asarray(bs, np.float64)
    Wout = np.asarray(Wout, np.float64)
    bout = np.asarray(bout, np.float64)

    A8 = _q8(Ws / 2.0)                       # fp8 scan weight
    shift = bs + bx + (Ws / 2.0).sum(axis=0)
    shift8 = _q8(shift)
    shiftc = _q8(16.0 * (shift - shift8.astype(np.float64)))
    Wx8 = _q8(Wx)
    Wo8 = _q8(Wout / 2.0)

    # scan moving operand [128, 4, 2, H] + input-projection one [128, 2, H]
    ws_l = np.zeros((128, 4, 2, H), F8NP)
    A8r = A8.reshape(KC, 128, H)
    for j in range(4):
        ws_l[:, j, 0] = A8r[2 * j]
        ws_l[:, j, 1] = A8r[2 * j + 1]
    wsx_l = np.zeros((128, 2, H), F8NP)
    wsx_l[0:I, 0] = Wx8
    wsx_l[I, 0] = shift8
    wsx_l[I + 1, 0] = shiftc
    wsx_l[I + 2, 0] = _q8(np.float32(-50.0))     # wall weight (~-48)

    # projection moving operand, direction-duplicated [128, KC, 2, O]
    wo_l = np.zeros((128, KC, 2, O), F8NP)
    Wo8r = Wo8.reshape(KC, 128, O)
    for kc in range(KC):
        wo_l[:, kc, 0] = Wo8r[kc]
        wo_l[:, kc, 1] = Wo8r[kc]

    bb = np.broadcast_to((Wout.sum(axis=0) + bout).astype(np.float32),
                         (128, O)).copy()

    # exact fold compensation: walled chains arrive at s = -1 exactly (fp8)
    A8cs = A8.astype(np.float64).sum(axis=0)

    x8 = _q8(x).astype(np.float32)
    s_idx = np.arange(S)
    in_maps = []
    for c in range(NCORES):
        xt = np.zeros((NSETS, S, 128, 2, 128), np.float32)
        fold = np.zeros((128, NSETS, H), np.float32)
        for e in range(NSETS):
            t0 = C * (NSETS * c + e)
            tf = t0 - W + s_idx                # fwd absolute times
            tb = t0 + (C - 1) + W - s_idx      # bwd absolute times
            ok_f = (tf >= 0) & (tf < T)
            ok_b = (tb >= 0) & (tb < T)
            xf = x8[:, np.clip(tf, 0, T - 1), :].transpose(1, 2, 0)
            xb = x8[:, np.clip(tb, 0, T - 1), :].transpose(1, 2, 0)
            xt[e, :, 0:I, 0, 0:64] = xf * ok_f[:, None, None]
            xt[e, :, 0:I, 0, 64:128] = xb * ok_b[:, None, None]
            xt[e, :, I, 0, :] = 1.0
            xt[e, :, I + 1, 0, :] = 0.0625
            # wall flags: only true-boundary chains' warmup steps
            if c == 0 and e == 0:
                xt[e, 0:W, I + 2, 0, 0:64] = 1.0
                fold[0:64, e] = ((2.0 * h0_f - 1.0) @ (Ws / 2.0) + A8cs)
            if c == NCORES - 1 and e == NSETS - 1:
                xt[e, 0:W, I + 2, 0, 64:128] = 1.0
                fold[64:128, e] = ((2.0 * h0_b - 1.0) @ (Ws / 2.0) + A8cs)

        in_maps.append({
            "xt": xt.astype(F8NP),
            "ws": ws_l,
            "wsx": wsx_l,
            "wo": wo_l,
            "fold": _round_fp22(fold),
            "bb": bb,
        })
    return in_maps


def _gather(results):
    full = np.empty((B, T, O), np.float32)
    for c in range(NCORES):
        o = results[c]["out"].reshape(NSETS, NP2, 2, 64, O)
        for e in range(NSETS):
            t0 = C * (NSETS * c + e)
            for p in range(NP2):
                # pair p holds times (HALF-1-p, HALF+p) in column halves
                full[:, t0 + HALF - 1 - p, :] = o[e, p, 0]
                full[:, t0 + HALF + p, :] = o[e, p, 1]
    return full


def kernel(x, h0_f, h0_b, Wx, bx, Ws, bs, Wout, bout):
    nc = _build_program()
    in_maps = _prepare_inputs(x, h0_f, h0_b, Wx, bx, Ws, bs, Wout, bout)
    res = run_bass_kernel_spmd(nc, in_maps, core_ids=list(range(NCORES)))
    return _gather(res.results)
